# revision 33
# baseline (speedup 1.0000x reference)
"""MultiHeadedAttention on 8 Trainium2 NeuronCores.

Sharding: sequence-sharded. Cores 0-3 handle batch 0, cores 4-7 batch 1.
Within a batch group, core c owns query AND key/value tokens [512c, 512c+512).
Each core projects Q/K/V for its own 512 tokens, K^T and augmented-V are
AllGathered within the 4-core group, attention runs over 512 queries x 2048
keys x 16 heads, and the output projection is local (no collective after).

Layouts (all device-side, transposes done on host):
  Q^T, K^T: [feature, token]  (contraction on partitions for matmuls)
  V:        [token, feature] augmented with a ones column per head ->
            PV matmul row 64 yields the softmax denominator for free.
  scores^T: [key, query]; softmax along partitions via the ones column.
  mask applied multiplicatively after exp (mask is 0/1).
"""

import numpy as np

B, S, D, H, DK = 2, 2048, 1024, 16, 64
NCORES = 8
GPB = 4            # cores per batch group
QB = S // GPB      # 512 tokens per core
NI = D // 128      # 8 feature chunks
NKB = S // 128     # 16 key blocks
NVC = H * (DK + 1)  # 1040 augmented V columns

TRACE = False
LAST_EXEC_NS = None
LAST_RESULTS = None
_PROG = None


def _build():
    from concourse import bass, mybir, tile

    FP = mybir.dt.float32
    FR = mybir.dt.float32r
    AF = mybir.ActivationFunctionType
    OP = mybir.AluOpType

    nc = bass.Bass(num_devices=NCORES)

    xq_t = nc.dram_tensor("xq_t", [D, QB], FP, kind="ExternalInput")
    xk_t = nc.dram_tensor("xk_t", [D, QB], FP, kind="ExternalInput")
    xv_t = nc.dram_tensor("xv_t", [D, QB], FP, kind="ExternalInput")
    mask_t = nc.dram_tensor("mask_t", [S, QB], FP, kind="ExternalInput")
    wq_t = nc.dram_tensor("wq_t", [D, D], FP, kind="ExternalInput")
    wk_t = nc.dram_tensor("wk_t", [D, D], FP, kind="ExternalInput")
    wv_t = nc.dram_tensor("wv_t", [D, D], FP, kind="ExternalInput")
    wo_t = nc.dram_tensor("wo_t", [D, D], FP, kind="ExternalInput")
    bq2 = nc.dram_tensor("bq2", [128, NI], FP, kind="ExternalInput")
    bk2 = nc.dram_tensor("bk2", [128, NI], FP, kind="ExternalInput")
    bv_b = nc.dram_tensor("bv_b", [128, D], FP, kind="ExternalInput")
    bo_b = nc.dram_tensor("bo_b", [128, D], FP, kind="ExternalInput")
    out = nc.dram_tensor("out", [QB, D], FP, kind="ExternalOutput")

    groups = [[0, 1, 2, 3], [4, 5, 6, 7]]

    with tile.TileContext(nc) as tc:
        with tc.tile_pool(name="dram", bufs=1, space="DRAM") as dpool, \
             tc.tile_pool(name="persist", bufs=1) as pp:

            kt_in = dpool.tile([D, QB], FP, tag="kt_in")
            v_in = dpool.tile([QB, NVC], FP, tag="v_in")
            kt_g = dpool.tile([GPB * D, QB], FP, tag="kt_g")
            v_g = dpool.tile([GPB * QB, NVC], FP, tag="v_g")

            qt = pp.tile([128, NI, QB], FR, tag="qt")
            at = pp.tile([128, NI, QB], FR, tag="at")
            msk = pp.tile([128, NKB, QB], FR, tag="msk")
            bq_sb = pp.tile([128, NI], FP, tag="bq")
            bk_sb = pp.tile([128, NI], FP, tag="bk")
            bv_bc = pp.tile([128, D], FP, tag="bv")
            bo_bc = pp.tile([128, D], FP, tag="bo")
            ones_c = pp.tile([1, 64], FP, tag="ones_c")

            # bias prep (bv_b/bo_b arrive pre-broadcast from host)
            nc.sync.dma_start(bq_sb[:], bq2[:])
            nc.sync.dma_start(bk_sb[:], bk2[:])
            nc.sync.dma_start(bv_bc[:], bv_b[:])
            nc.sync.dma_start(bo_bc[:], bo_b[:])
            nc.gpsimd.memset(ones_c[:], 1.0)

            # mask load
            for kb in range(NKB):
                nc.sync.dma_start(msk[:, kb, :],
                                  mask_t[kb * 128:(kb + 1) * 128, :].bitcast(FR))

            # ---------------- projection phase ----------------
            with tc.tile_pool(name="wstage", bufs=2) as wp, \
                 tc.tile_pool(name="xstage", bufs=2) as xp, \
                 tc.tile_pool(name="kst", bufs=2) as kstp, \
                 tc.tile_pool(name="vst", bufs=2) as vstp, \
                 tc.tile_pool(name="psA", bufs=3, space="PSUM") as psA:

                def load_x(src):
                    x_sb = xp.tile([128, NI, QB], FR, name="x_sb")
                    for ci in range(NI):
                        nc.sync.dma_start(x_sb[:, ci, :],
                                          src[ci * 128:(ci + 1) * 128, :].bitcast(FR))
                    return x_sb

                def load_w_half(src, oh):
                    w_sb = wp.tile([128, NI, 512], FR, name="w_sb")
                    for ci in range(NI):
                        nc.sync.dma_start(
                            w_sb[:, ci, :],
                            src[ci * 128:(ci + 1) * 128,
                                oh * 512:(oh + 1) * 512].bitcast(FR))
                    return w_sb

                # K projection -> kt_in DRAM (feature-major), then AllGather
                xk_sb = load_x(xk_t)
                for oh in range(2):
                    wk_sb = load_w_half(wk_t, oh)
                    for obl in range(4):
                        ob = oh * 4 + obl
                        ps = psA.tile([128, QB], FP, name="ps")
                        for ci in range(NI):
                            nc.tensor.matmul(
                                ps[:],
                                lhsT=wk_sb[:, ci, obl * 128:(obl + 1) * 128],
                                rhs=xk_sb[:, ci, :],
                                start=(ci == 0), stop=(ci == NI - 1))
                        kstage = kstp.tile([128, QB], FP, name="kstage")
                        ps_bc, bk_bc = bass.broadcast_tensor_aps(
                            ps[:], bk_sb[:, ob:ob + 1])
                        nc.vector.tensor_tensor(kstage[:], ps_bc, bk_bc,
                                                mybir.AluOpType.add)
                        nc.sync.dma_start(kt_in[ob * 128:(ob + 1) * 128, :],
                                          kstage[:])
                nc.gpsimd.collective_compute(
                    "AllGather", mybir.AluOpType.bypass,
                    replica_groups=groups, ins=[kt_in[:]], outs=[kt_g[:]])

                # V projection -> v_in DRAM (token-major, ones-augmented)
                xv_sb = load_x(xv_t)
                wv_sbs = [load_w_half(wv_t, oh) for oh in range(2)]
                for tb in range(4):
                    vstage = vstp.tile([128, H, DK + 1], FP, name="vstage")
                    nc.gpsimd.memset(vstage[:, :, DK:DK + 1], 1.0)
                    for oh in range(2):
                        ps = psA.tile([128, 512], FP, name="ps")
                        for ci in range(NI):
                            nc.tensor.matmul(
                                ps[:],
                                lhsT=xv_sb[:, ci, tb * 128:(tb + 1) * 128],
                                rhs=wv_sbs[oh][:, ci, :],
                                start=(ci == 0), stop=(ci == NI - 1))
                        for hh in range(8):
                            h = oh * 8 + hh
                            nc.vector.tensor_tensor(
                                vstage[:, h, 0:DK],
                                ps[:, hh * DK:(hh + 1) * DK],
                                bv_bc[:, h * DK:(h + 1) * DK],
                                mybir.AluOpType.add)
                    nc.sync.dma_start(v_in[tb * 128:(tb + 1) * 128, :],
                                      vstage[:].opt())
                nc.gpsimd.collective_compute(
                    "AllGather", mybir.AluOpType.bypass,
                    replica_groups=groups, ins=[v_in[:]], outs=[v_g[:]])

                # Q projection -> qt SBUF (feature-major), stays local
                xq_sb = load_x(xq_t)
                for oh in range(2):
                    wq_sb = load_w_half(wq_t, oh)
                    for obl in range(4):
                        ob = oh * 4 + obl
                        ps = psA.tile([128, QB], FP, name="ps")
                        for ci in range(NI):
                            nc.tensor.matmul(
                                ps[:],
                                lhsT=wq_sb[:, ci, obl * 128:(obl + 1) * 128],
                                rhs=xq_sb[:, ci, :],
                                start=(ci == 0), stop=(ci == NI - 1))
                        ps_bc, bq_bc = bass.broadcast_tensor_aps(
                            ps[:], bq_sb[:, ob:ob + 1])
                        nc.vector.tensor_tensor(qt[:, ob, :], ps_bc, bq_bc,
                                                mybir.AluOpType.add)

            # ---------------- attention + output phase ----------------
            with tc.tile_pool(name="wo", bufs=1) as wop:
                vaug = wop.tile([128, NKB, H, DK + 1], FR, tag="vaug")
                wo_sb = wop.tile([128, NI, D], FR, tag="wo_sb")
                for ci in range(NI):
                    nc.scalar.dma_start(wo_sb[:, ci, :],
                                        wo_t[ci * 128:(ci + 1) * 128, :].bitcast(FR))

                # V gather readback into SBUF (all ranks, incl. own)
                for kc in range(NKB):
                    nc.sync.dma_start(vaug[:, kc, :, :].opt(),
                                      v_g[kc * 128:(kc + 1) * 128, :].bitcast(FR))

                with tc.tile_pool(name="kstream", bufs=4) as kp, \
                     tc.tile_pool(name="probs", bufs=3) as pbp, \
                     tc.tile_pool(name="rb", bufs=2) as rbp, \
                     tc.tile_pool(name="scps", bufs=2, space="PSUM") as scp, \
                     tc.tile_pool(name="rbps", bufs=2, space="PSUM") as rbps, \
                     tc.tile_pool(name="pvps", bufs=2, space="PSUM") as pvp:

                    for h in range(H):
                        ci_h = h // 2
                        p0 = 64 * (h % 2)
                        q_ap = qt[p0:p0 + 64, ci_h, :]
                        pv = pvp.tile([DK + 1, QB], FP, name="pv")
                        for kb2 in range(NKB // 2):
                            rk = kb2 // 2
                            l0 = (kb2 % 2) * 256
                            kstr = kp.tile([128, 256], FR, name="kstr")
                            row = rk * D + ci_h * 128 + p0
                            nc.sync.dma_start(kstr[p0:p0 + 64, :],
                                              kt_g[row:row + 64,
                                                   l0:l0 + 256].bitcast(FR))
                            sc = scp.tile([128, 2, 512], FP, name="sc")
                            for j in range(2):
                                nc.tensor.matmul(
                                    sc[:, j, :],
                                    lhsT=kstr[p0:p0 + 64,
                                              j * 128:(j + 1) * 128],
                                    rhs=q_ap,
                                    start=True, stop=True)
                            pb = pbp.tile([128, 2, 512], FR, name="pb")
                            nc.scalar.activation(pb[:], sc[:], AF.Exp, scale=0.125)
                            nc.vector.tensor_tensor(
                                pb[:], pb[:], msk[:, 2 * kb2:2 * kb2 + 2, :],
                                mybir.AluOpType.mult)
                            for j in range(2):
                                kb = kb2 * 2 + j
                                nc.tensor.matmul(
                                    pv[:],
                                    lhsT=vaug[:, kb, h, :],
                                    rhs=pb[:, j, :],
                                    start=(kb == 0), stop=(kb == NKB - 1))
                        nc.vector.tensor_copy(at[p0:p0 + 64, ci_h, :], pv[0:DK, :])
                        rden = rbp.tile([1, QB], FP, name="rden")
                        nc.vector.reciprocal(rden[0:1, :], pv[DK:DK + 1, :])
                        # broadcast 1/denom across 64 partitions via K=1 matmul
                        rbq = rbps.tile([128, QB], FP, name="rbq")
                        nc.tensor.matmul(rbq[p0:p0 + 64, :],
                                         lhsT=ones_c[0:1, :],
                                         rhs=rden[0:1, :],
                                         start=True, stop=True)
                        nc.vector.tensor_tensor(
                            at[p0:p0 + 64, ci_h, :], at[p0:p0 + 64, ci_h, :],
                            rbq[p0:p0 + 64, :], mybir.AluOpType.mult)

                # output projection: out[q, o] with A^T stationary
                with tc.tile_pool(name="outsb", bufs=2) as osp, \
                     tc.tile_pool(name="pops", bufs=2, space="PSUM") as pop:
                    for qb in range(4):
                        osb = osp.tile([128, D], FP, name="osb")
                        for oh in range(2):
                            po = pop.tile([128, 512], FP, name="po")
                            for ci in range(NI):
                                nc.tensor.matmul(
                                    po[:],
                                    lhsT=at[:, ci, qb * 128:(qb + 1) * 128],
                                    rhs=wo_sb[:, ci, oh * 512:(oh + 1) * 512],
                                    start=(ci == 0), stop=(ci == NI - 1))
                            nc.vector.tensor_tensor(
                                osb[:, oh * 512:(oh + 1) * 512], po[:],
                                bo_bc[:, oh * 512:(oh + 1) * 512],
                                mybir.AluOpType.add)
                        nc.sync.dma_start(out[qb * 128:(qb + 1) * 128, :], osb[:])
    return nc


_PATCHED = False


def _split_multi_waits(bir_bytes):
    # This walrus build allows only one sync-wait command per instruction.
    # Hoist extra waits onto EventSemaphore carriers just before each
    # instruction in the same engine stream (engines execute in order).
    import json
    j = json.loads(bir_bytes)
    for fn in j["functions"]:
        for blk in fn["blocks"]:
            out = []
            for inst in blk["instructions"]:
                si = inst.get("sync_info") or {}
                waits = si.get("on_wait") or []
                if len(waits) > 1:
                    for k, w in enumerate(waits[:-1]):
                        out.append({
                            "debug": inst.get("debug", 0),
                            "engine": inst["engine"],
                            "ins": [],
                            "name": f"{inst['name']}_w{k}",
                            "opcode": "EventSemaphore",
                            "outs": [],
                            "sync_info": {"on_update": [], "on_wait": [w]},
                        })
                    si["on_wait"] = [waits[-1]]
                out.append(inst)
            blk["instructions"] = out
    return json.dumps(j).encode()


def _patch_compiler():
    global _PATCHED
    if _PATCHED:
        return
    from concourse import bass_utils, bass2jax
    orig = bass_utils.compile_bir_kernel

    def wrapped(bir_json, tmpdir, neff_name="file.neff"):
        return orig(_split_multi_waits(bir_json), tmpdir, neff_name)

    bass_utils.compile_bir_kernel = wrapped
    bass2jax.compile_bir_kernel = wrapped
    _PATCHED = True


def kernel(query, key, value, mask, Wq, bq, Wk, bk, Wv, bv, Wo, bo):
    global LAST_EXEC_NS, LAST_RESULTS, _PROG
    _patch_compiler()
    from concourse.bass_utils import run_bass_kernel_spmd

    f32 = np.float32
    wq_t = np.ascontiguousarray(np.asarray(Wq, dtype=f32).T)
    wk_t = np.ascontiguousarray(np.asarray(Wk, dtype=f32).T)
    wv_t = np.ascontiguousarray(np.asarray(Wv, dtype=f32).T)
    wo_t = np.ascontiguousarray(np.asarray(Wo, dtype=f32).T)
    bq2 = np.ascontiguousarray(np.asarray(bq, dtype=f32).reshape(NI, 128).T)
    bk2 = np.ascontiguousarray(np.asarray(bk, dtype=f32).reshape(NI, 128).T)
    bv_b = np.ascontiguousarray(
        np.broadcast_to(np.asarray(bv, dtype=f32).reshape(1, D), (128, D)))
    bo_b = np.ascontiguousarray(
        np.broadcast_to(np.asarray(bo, dtype=f32).reshape(1, D), (128, D)))

    in_maps = []
    for r in range(NCORES):
        b, c = divmod(r, GPB)
        q0 = QB * c
        in_maps.append({
            "xq_t": np.ascontiguousarray(
                np.asarray(query[b, q0:q0 + QB, :], dtype=f32).T),
            "xk_t": np.ascontiguousarray(
                np.asarray(key[b, q0:q0 + QB, :], dtype=f32).T),
            "xv_t": np.ascontiguousarray(
                np.asarray(value[b, q0:q0 + QB, :], dtype=f32).T),
            "mask_t": np.ascontiguousarray(
                np.asarray(mask[b, q0:q0 + QB, :], dtype=f32).T),
            "wq_t": wq_t, "wk_t": wk_t, "wv_t": wv_t, "wo_t": wo_t,
            "bq2": bq2, "bk2": bk2, "bv_b": bv_b, "bo_b": bo_b,
        })

    if _PROG is None:
        _PROG = _build()

    res = run_bass_kernel_spmd(_PROG, in_maps, core_ids=list(range(NCORES)),
                               trace=TRACE)
    LAST_EXEC_NS = res.exec_time_ns
    LAST_RESULTS = res

    out_full = np.empty((B, S, D), dtype=f32)
    for r in range(NCORES):
        b, c = divmod(r, GPB)
        q0 = QB * c
        out_full[b, q0:q0 + QB, :] = res.results[r]["out"]
    return out_full


# revision 42
# speedup vs baseline: 1.0174x; 1.0174x over previous
"""MultiHeadedAttention on 8 Trainium2 NeuronCores.

Sharding: sequence-sharded. Cores 0-3 handle batch 0, cores 4-7 batch 1.
Within a batch group, core c owns query AND key/value tokens [512c, 512c+512).
Each core projects Q/K/V for its own 512 tokens, K^T and augmented-V are
AllGathered within the 4-core group, attention runs over 512 queries x 2048
keys x 16 heads, and the output projection is local (no collective after).

Layouts (all device-side, transposes done on host):
  Q^T, K^T: [feature, token]  (contraction on partitions for matmuls)
  V:        [token, feature] augmented with a ones column per head ->
            PV matmul row 64 yields the softmax denominator for free.
  scores^T: [key, query]; softmax along partitions via the ones column.
  mask applied multiplicatively after exp (mask is 0/1).
"""

import numpy as np

B, S, D, H, DK = 2, 2048, 1024, 16, 64
NCORES = 8
GPB = 4            # cores per batch group
QB = S // GPB      # 512 tokens per core
NI = D // 128      # 8 feature chunks
NKB = S // 128     # 16 key blocks
NVC = H * (DK + 1)  # 1040 augmented V columns

TRACE = False
LAST_EXEC_NS = None
LAST_RESULTS = None
_PROG = None


def _build():
    from concourse import bass, mybir, tile

    FP = mybir.dt.float32
    FR = mybir.dt.float32r
    BF = mybir.dt.bfloat16
    AF = mybir.ActivationFunctionType
    OP = mybir.AluOpType

    nc = bass.Bass(num_devices=NCORES)

    xq_t = nc.dram_tensor("xq_t", [D, QB], FP, kind="ExternalInput")
    xk_t = nc.dram_tensor("xk_t", [D, QB], FP, kind="ExternalInput")
    xv_t = nc.dram_tensor("xv_t", [D, QB], FP, kind="ExternalInput")
    mask_t = nc.dram_tensor("mask_t", [S, QB], FP, kind="ExternalInput")
    wq_t = nc.dram_tensor("wq_t", [D, D], FP, kind="ExternalInput")
    wk_t = nc.dram_tensor("wk_t", [D, D], FP, kind="ExternalInput")
    wv_t = nc.dram_tensor("wv_t", [D, D], FP, kind="ExternalInput")
    wo_t = nc.dram_tensor("wo_t", [D, D], FP, kind="ExternalInput")
    bq2 = nc.dram_tensor("bq2", [128, NI], FP, kind="ExternalInput")
    bk2 = nc.dram_tensor("bk2", [128, NI], FP, kind="ExternalInput")
    bv_b = nc.dram_tensor("bv_b", [128, D], FP, kind="ExternalInput")
    bo_b = nc.dram_tensor("bo_b", [128, D], FP, kind="ExternalInput")
    out = nc.dram_tensor("out", [QB, D], FP, kind="ExternalOutput")

    groups = [[0, 1, 2, 3], [4, 5, 6, 7]]

    with tile.TileContext(nc) as tc:
        with tc.tile_pool(name="dram", bufs=1, space="DRAM") as dpool, \
             tc.tile_pool(name="persist", bufs=1) as pp:

            kt_in = dpool.tile([D, QB], BF, tag="kt_in")
            v_in = dpool.tile([QB, NVC], FP, tag="v_in")
            kt_g = dpool.tile([GPB * D, QB], BF, tag="kt_g")
            v_g = dpool.tile([GPB * QB, NVC], FP, tag="v_g")

            qt = pp.tile([128, NI, QB], BF, tag="qt")
            at = pp.tile([128, NI, QB], FR, tag="at")
            msk = pp.tile([128, NKB, QB], FR, tag="msk")
            bq_sb = pp.tile([128, NI], FP, tag="bq")
            bk_sb = pp.tile([128, NI], FP, tag="bk")
            bv_bc = pp.tile([128, D], FP, tag="bv")
            bo_bc = pp.tile([128, D], FP, tag="bo")
            ones_c = pp.tile([1, 64], FP, tag="ones_c")

            # bias prep (bv_b/bo_b arrive pre-broadcast from host)
            nc.sync.dma_start(bq_sb[:], bq2[:])
            nc.sync.dma_start(bk_sb[:], bk2[:])
            nc.sync.dma_start(bv_bc[:], bv_b[:])
            nc.sync.dma_start(bo_bc[:], bo_b[:])
            nc.gpsimd.memset(ones_c[:], 1.0)

            # mask load
            for kb in range(NKB):
                nc.sync.dma_start(msk[:, kb, :],
                                  mask_t[kb * 128:(kb + 1) * 128, :].bitcast(FR))

            # ---------------- projection phase ----------------
            with tc.tile_pool(name="wstage", bufs=2) as wp, \
                 tc.tile_pool(name="xstage", bufs=2) as xp, \
                 tc.tile_pool(name="kst", bufs=2) as kstp, \
                 tc.tile_pool(name="vst", bufs=2) as vstp, \
                 tc.tile_pool(name="psA", bufs=3, space="PSUM") as psA:

                def load_x(src):
                    x_sb = xp.tile([128, NI, QB], FR, name="x_sb")
                    for ci in range(NI):
                        nc.sync.dma_start(x_sb[:, ci, :],
                                          src[ci * 128:(ci + 1) * 128, :].bitcast(FR))
                    return x_sb

                def load_w_half(src, oh):
                    w_sb = wp.tile([128, NI, 512], FR, name="w_sb")
                    for ci in range(NI):
                        nc.sync.dma_start(
                            w_sb[:, ci, :],
                            src[ci * 128:(ci + 1) * 128,
                                oh * 512:(oh + 1) * 512].bitcast(FR))
                    return w_sb

                # K projection -> kt_in DRAM (feature-major), then AllGather
                xk_sb = load_x(xk_t)
                for oh in range(2):
                    wk_sb = load_w_half(wk_t, oh)
                    for obl in range(4):
                        ob = oh * 4 + obl
                        ps = psA.tile([128, QB], FP, name="ps")
                        for ci in range(NI):
                            nc.tensor.matmul(
                                ps[:],
                                lhsT=wk_sb[:, ci, obl * 128:(obl + 1) * 128],
                                rhs=xk_sb[:, ci, :],
                                start=(ci == 0), stop=(ci == NI - 1))
                        kstage = kstp.tile([128, QB], BF, name="kstage")
                        ps_bc, bk_bc = bass.broadcast_tensor_aps(
                            ps[:], bk_sb[:, ob:ob + 1])
                        nc.vector.tensor_tensor(kstage[:], ps_bc, bk_bc,
                                                mybir.AluOpType.add)
                        nc.sync.dma_start(kt_in[ob * 128:(ob + 1) * 128, :],
                                          kstage[:])
                nc.gpsimd.collective_compute(
                    "AllGather", mybir.AluOpType.bypass,
                    replica_groups=groups, ins=[kt_in[:]], outs=[kt_g[:]])

                # V projection -> v_in DRAM (token-major, ones-augmented)
                xv_sb = load_x(xv_t)
                wv_sbs = [load_w_half(wv_t, oh) for oh in range(2)]
                for tb in range(4):
                    vstage = vstp.tile([128, H, DK + 1], FP, name="vstage")
                    nc.gpsimd.memset(vstage[:, :, DK:DK + 1], 1.0)
                    for oh in range(2):
                        ps = psA.tile([128, 512], FP, name="ps")
                        for ci in range(NI):
                            nc.tensor.matmul(
                                ps[:],
                                lhsT=xv_sb[:, ci, tb * 128:(tb + 1) * 128],
                                rhs=wv_sbs[oh][:, ci, :],
                                start=(ci == 0), stop=(ci == NI - 1))
                        for hh in range(8):
                            h = oh * 8 + hh
                            nc.vector.tensor_tensor(
                                vstage[:, h, 0:DK],
                                ps[:, hh * DK:(hh + 1) * DK],
                                bv_bc[:, h * DK:(h + 1) * DK],
                                mybir.AluOpType.add)
                    nc.sync.dma_start(v_in[tb * 128:(tb + 1) * 128, :],
                                      vstage[:].opt())
                nc.gpsimd.collective_compute(
                    "AllGather", mybir.AluOpType.bypass,
                    replica_groups=groups, ins=[v_in[:]], outs=[v_g[:]])

                # Q projection -> qt SBUF (feature-major), stays local
                xq_sb = load_x(xq_t)
                for oh in range(2):
                    wq_sb = load_w_half(wq_t, oh)
                    for obl in range(4):
                        ob = oh * 4 + obl
                        ps = psA.tile([128, QB], FP, name="ps")
                        for ci in range(NI):
                            nc.tensor.matmul(
                                ps[:],
                                lhsT=wq_sb[:, ci, obl * 128:(obl + 1) * 128],
                                rhs=xq_sb[:, ci, :],
                                start=(ci == 0), stop=(ci == NI - 1))
                        ps_bc, bq_bc = bass.broadcast_tensor_aps(
                            ps[:], bq_sb[:, ob:ob + 1])
                        nc.vector.tensor_tensor(qt[:, ob, :], ps_bc, bq_bc,
                                                mybir.AluOpType.add)

            # ---------------- attention + output phase ----------------
            with tc.tile_pool(name="wo", bufs=1) as wop:
                vaug = wop.tile([128, NKB, H, DK + 1], FR, tag="vaug")
                wo_sb = wop.tile([128, NI, D], FR, tag="wo_sb")
                for ci in range(NI):
                    nc.scalar.dma_start(wo_sb[:, ci, :],
                                        wo_t[ci * 128:(ci + 1) * 128, :].bitcast(FR))

                # V gather readback into SBUF (all ranks, incl. own)
                for kc in range(NKB):
                    nc.sync.dma_start(vaug[:, kc, :, :].opt(),
                                      v_g[kc * 128:(kc + 1) * 128, :].bitcast(FR))

                with tc.tile_pool(name="kstream", bufs=4) as kp, \
                     tc.tile_pool(name="probs", bufs=3) as pbp, \
                     tc.tile_pool(name="rb", bufs=2) as rbp, \
                     tc.tile_pool(name="scps", bufs=2, space="PSUM") as scp, \
                     tc.tile_pool(name="rbps", bufs=2, space="PSUM") as rbps, \
                     tc.tile_pool(name="pvps", bufs=2, space="PSUM") as pvp:

                    for h in range(H):
                        ci_h = h // 2
                        p0 = 64 * (h % 2)
                        q_ap = qt[p0:p0 + 64, ci_h, :]
                        pv = pvp.tile([DK + 1, QB], FP, name="pv")
                        for kb2 in range(NKB // 2):
                            rk = kb2 // 2
                            l0 = (kb2 % 2) * 256
                            kstr = kp.tile([128, 256], BF, name="kstr")
                            row = rk * D + ci_h * 128 + p0
                            nc.sync.dma_start(kstr[p0:p0 + 64, :],
                                              kt_g[row:row + 64, l0:l0 + 256])
                            sc = scp.tile([128, 2, 512], FP, name="sc")
                            for j in range(2):
                                nc.tensor.matmul(
                                    sc[:, j, :],
                                    lhsT=kstr[p0:p0 + 64,
                                              j * 128:(j + 1) * 128],
                                    rhs=q_ap,
                                    start=True, stop=True)
                            pb = pbp.tile([128, 2, 512], FR, name="pb")
                            nc.scalar.activation(pb[:], sc[:], AF.Exp, scale=0.125)
                            nc.vector.tensor_tensor(
                                pb[:], pb[:], msk[:, 2 * kb2:2 * kb2 + 2, :],
                                mybir.AluOpType.mult)
                            for j in range(2):
                                kb = kb2 * 2 + j
                                nc.tensor.matmul(
                                    pv[:],
                                    lhsT=vaug[:, kb, h, :],
                                    rhs=pb[:, j, :],
                                    start=(kb == 0), stop=(kb == NKB - 1))
                        nc.vector.tensor_copy(at[p0:p0 + 64, ci_h, :], pv[0:DK, :])
                        rden = rbp.tile([1, QB], FP, name="rden")
                        nc.vector.reciprocal(rden[0:1, :], pv[DK:DK + 1, :])
                        # broadcast 1/denom across 64 partitions via K=1 matmul
                        rbq = rbps.tile([128, QB], FP, name="rbq")
                        nc.tensor.matmul(rbq[p0:p0 + 64, :],
                                         lhsT=ones_c[0:1, :],
                                         rhs=rden[0:1, :],
                                         start=True, stop=True)
                        nc.vector.tensor_tensor(
                            at[p0:p0 + 64, ci_h, :], at[p0:p0 + 64, ci_h, :],
                            rbq[p0:p0 + 64, :], mybir.AluOpType.mult)

                # output projection: out[q, o] with A^T stationary
                with tc.tile_pool(name="outsb", bufs=2) as osp, \
                     tc.tile_pool(name="pops", bufs=2, space="PSUM") as pop:
                    for qb in range(4):
                        osb = osp.tile([128, D], FP, name="osb")
                        for oh in range(2):
                            po = pop.tile([128, 512], FP, name="po")
                            for ci in range(NI):
                                nc.tensor.matmul(
                                    po[:],
                                    lhsT=at[:, ci, qb * 128:(qb + 1) * 128],
                                    rhs=wo_sb[:, ci, oh * 512:(oh + 1) * 512],
                                    start=(ci == 0), stop=(ci == NI - 1))
                            nc.vector.tensor_tensor(
                                osb[:, oh * 512:(oh + 1) * 512], po[:],
                                bo_bc[:, oh * 512:(oh + 1) * 512],
                                mybir.AluOpType.add)
                        nc.sync.dma_start(out[qb * 128:(qb + 1) * 128, :], osb[:])
    return nc


_PATCHED = False


def _split_multi_waits(bir_bytes):
    # This walrus build allows only one sync-wait command per instruction.
    # Hoist extra waits onto EventSemaphore carriers just before each
    # instruction in the same engine stream (engines execute in order).
    import json
    j = json.loads(bir_bytes)
    for fn in j["functions"]:
        for blk in fn["blocks"]:
            out = []
            for inst in blk["instructions"]:
                si = inst.get("sync_info") or {}
                waits = si.get("on_wait") or []
                if len(waits) > 1:
                    for k, w in enumerate(waits[:-1]):
                        out.append({
                            "debug": inst.get("debug", 0),
                            "engine": inst["engine"],
                            "ins": [],
                            "name": f"{inst['name']}_w{k}",
                            "opcode": "EventSemaphore",
                            "outs": [],
                            "sync_info": {"on_update": [], "on_wait": [w]},
                        })
                    si["on_wait"] = [waits[-1]]
                out.append(inst)
            blk["instructions"] = out
    return json.dumps(j).encode()


def _patch_compiler():
    global _PATCHED
    if _PATCHED:
        return
    from concourse import bass_utils, bass2jax
    orig = bass_utils.compile_bir_kernel

    def wrapped(bir_json, tmpdir, neff_name="file.neff"):
        return orig(_split_multi_waits(bir_json), tmpdir, neff_name)

    bass_utils.compile_bir_kernel = wrapped
    bass2jax.compile_bir_kernel = wrapped
    _PATCHED = True


def kernel(query, key, value, mask, Wq, bq, Wk, bk, Wv, bv, Wo, bo):
    global LAST_EXEC_NS, LAST_RESULTS, _PROG
    _patch_compiler()
    from concourse.bass_utils import run_bass_kernel_spmd

    f32 = np.float32
    wq_t = np.ascontiguousarray(np.asarray(Wq, dtype=f32).T)
    wk_t = np.ascontiguousarray(np.asarray(Wk, dtype=f32).T)
    wv_t = np.ascontiguousarray(np.asarray(Wv, dtype=f32).T)
    wo_t = np.ascontiguousarray(np.asarray(Wo, dtype=f32).T)
    bq2 = np.ascontiguousarray(np.asarray(bq, dtype=f32).reshape(NI, 128).T)
    bk2 = np.ascontiguousarray(np.asarray(bk, dtype=f32).reshape(NI, 128).T)
    bv_b = np.ascontiguousarray(
        np.broadcast_to(np.asarray(bv, dtype=f32).reshape(1, D), (128, D)))
    bo_b = np.ascontiguousarray(
        np.broadcast_to(np.asarray(bo, dtype=f32).reshape(1, D), (128, D)))

    in_maps = []
    for r in range(NCORES):
        b, c = divmod(r, GPB)
        q0 = QB * c
        in_maps.append({
            "xq_t": np.ascontiguousarray(
                np.asarray(query[b, q0:q0 + QB, :], dtype=f32).T),
            "xk_t": np.ascontiguousarray(
                np.asarray(key[b, q0:q0 + QB, :], dtype=f32).T),
            "xv_t": np.ascontiguousarray(
                np.asarray(value[b, q0:q0 + QB, :], dtype=f32).T),
            "mask_t": np.ascontiguousarray(
                np.asarray(mask[b, q0:q0 + QB, :], dtype=f32).T),
            "wq_t": wq_t, "wk_t": wk_t, "wv_t": wv_t, "wo_t": wo_t,
            "bq2": bq2, "bk2": bk2, "bv_b": bv_b, "bo_b": bo_b,
        })

    if _PROG is None:
        _PROG = _build()

    res = run_bass_kernel_spmd(_PROG, in_maps, core_ids=list(range(NCORES)),
                               trace=TRACE)
    LAST_EXEC_NS = res.exec_time_ns
    LAST_RESULTS = res

    out_full = np.empty((B, S, D), dtype=f32)
    for r in range(NCORES):
        b, c = divmod(r, GPB)
        q0 = QB * c
        out_full[b, q0:q0 + QB, :] = res.results[r]["out"]
    return out_full
